# revision 1
# baseline (speedup 1.0000x reference)
"""Trainium2 Bass kernel for nn_C_T_F_Attention_90529320665770.

Math (per reference.py):
  Each branch (c,f,t) does conv1x1+BN on q,k then means over the output
  channel axis.  BN is a per-channel affine, so
     mean_o BN(W @ x)_o = ((1/O) * s @ W) . x + mean(t)  =: a . x + tbar
  i.e. each branch's q,k collapse to a single length-Cin contraction.
  logits = outer(qv, kv); softmax over j of  u_i * kv_j  with
  u = scale_l * (qv + tbar_q); the j-constant terms cancel in softmax.
  Needed output is only the softmax diagonal summed over rows:
     d[i] = sum_n exp(u_i kv_i) / sum_j exp(u_i kv_j)
  |u_i kv_j| is small, so  sum_j exp(u_i kv_j) = sum_p u_i^p/p! * S_p,
  S_p = sum_j kv_j^p  (Taylor-moment trick, degree 8, exact to ~1e-9 here).
  Final: out = v * (dc[c] + dt[f] + df[t]) + x,  v = BN(v_w @ x + v_b).

Sharding: pure data-parallel over batch B=32 across 8 cores (4 each);
the only coupling is an AllReduce of the 514 floats (dc,df,dt).
"""
import sys
sys.path.insert(0, '/opt/trn_rl_repo')

import numpy as np
import ml_dtypes

import concourse.bass as bass
import concourse.tile as tile
from concourse import bacc, mybir
from concourse.bass_utils import run_bass_kernel_spmd

EPS = 1e-5
C, FR, T = 256, 7, 251
B = 32
N_CORES = 8
BPC = B // N_CORES              # batches per core = 4
NFT = FR * T                    # 1757
NCOLS = BPC * NFT               # 7028  (b,f,t) columns per core
NG = BPC * FR                   # 28 (b,f) groups per core
NC_COLS = NG * C                # 7168  (b,f,c) columns for branch c
P = 4                           # taylor degree
VMM_SPLIT3 = True               # 3-term bf16-split v-matmul (~1e-5) vs fp32r (~1.6e-4) (powers 1..P, S_0 analytic)

f32 = mybir.dt.float32
f32r = mybir.dt.float32r
bf16 = mybir.dt.bfloat16
FT = mybir.ActivationFunctionType
ALU = mybir.AluOpType
AX = mybir.AxisListType

_FACT = [1.0, 1.0, 2.0, 6.0, 24.0, 120.0, 720.0, 5040.0, 40320.0]


def _bn_fold(bn):
    g, b_, m, v = bn.astype(np.float64)
    s = g / np.sqrt(v + EPS)
    t = b_ - m * s
    return s, t


def _branch_fold(qw, qbn, kw, kbn, lbn):
    s_q, t_q = _bn_fold(qbn)
    s_k, _ = _bn_fold(kbn)
    o = qw.shape[0]
    a_q = (s_q @ qw.astype(np.float64)) / o
    tq = t_q.mean()
    a_k = (s_k @ kw.astype(np.float64)) / o
    gl, bl, ml, vl = lbn[:, 0].astype(np.float64)
    scale_l = gl / np.sqrt(vl + EPS)
    return (scale_l * a_q).astype(np.float32), np.float32(scale_l * tq), \
        a_k.astype(np.float32)


def _build_program():
    nc = bacc.Bacc("TRN2", target_bir_lowering=False, debug=False,
                   num_devices=N_CORES)

    # ---- per-core DRAM I/O ----
    x_in = nc.declare_dram_parameter("x_in", [BPC, C, NFT], f32r, isOutput=False)
    xct_in = nc.declare_dram_parameter("xct_in", [T, NC_COLS], bf16, isOutput=False)
    xsb_in = nc.declare_dram_parameter("xsb_in", [C, NCOLS], bf16, isOutput=False)
    aft_in = nc.declare_dram_parameter("aft_in", [C, 4], bf16, isOutput=False)
    ac_in = nc.declare_dram_parameter("ac_in", [T, 2], bf16, isOutput=False)
    vwT_in = nc.declare_dram_parameter("vwT_in", [C, C], f32r, isOutput=False)
    if VMM_SPLIT3:
        whi_in = nc.declare_dram_parameter("whi_in", [C, C], bf16, isOutput=False)
        wlo_in = nc.declare_dram_parameter("wlo_in", [C, C], bf16, isOutput=False)
        xlo_in = nc.declare_dram_parameter("xlo_in", [C, NCOLS], bf16, isOutput=False)
    sv_in = nc.declare_dram_parameter("sv_in", [128, 2], f32, isOutput=False)
    tv_in = nc.declare_dram_parameter("tv_in", [128, 2], f32, isOutput=False)
    tq_in = nc.declare_dram_parameter("tq_in", [1, 4], f32, isOutput=False)
    mask_in = nc.declare_dram_parameter("mask_in", [128, 2], f32, isOutput=False)
    out_d = nc.declare_dram_parameter("out", [BPC, C, NFT], f32, isOutput=True)

    with tile.TileContext(nc) as tc:
        import contextlib
        ctx = contextlib.ExitStack()
        with ctx:
            pool = ctx.enter_context(tc.tile_pool(name="sb", bufs=1))
            psum = ctx.enter_context(tc.tile_pool(name="ps", bufs=1, space="PSUM"))
            psv = ctx.enter_context(tc.tile_pool(name="psv", bufs=4, space="PSUM"))
            dram = ctx.enter_context(tc.tile_pool(name="dr", bufs=1, space="DRAM"))

            # ---------- loads ----------
            xct = [pool.tile([128, NC_COLS], bf16, tag=f"xct{i}", name=f"xct{i}") for i in range(2)]

            ac = [pool.tile([128, 2], bf16, tag=f"ac{i}", name=f"ac{i}") for i in range(2)]
            nc.sync.dma_start(ac[0][:, :], ac_in[0:128, :])
            nc.sync.dma_start(ac[1][0:T - 128, :], ac_in[128:T, :])

            aft = [pool.tile([128, 4], bf16, tag=f"aft{i}", name=f"aft{i}") for i in range(2)]
            for k in range(2):
                nc.sync.dma_start(aft[k][:, :], aft_in[k * 128:(k + 1) * 128, :])
            xsb = [pool.tile([128, NCOLS], bf16, tag=f"xsb{i}", name=f"xsb{i}") for i in range(2)]

            xs = [pool.tile([128, NCOLS], f32r, tag=f"xs{i}", name=f"xs{i}") for i in range(2)]

            vwT = None if VMM_SPLIT3 else [pool.tile([128, 128], f32r, tag=f"vw{i}", name=f"vw{i}") for i in range(4)]

            sv = pool.tile([128, 2], f32, tag="sv")
            tv = pool.tile([128, 2], f32, tag="tv")
            nc.sync.dma_start(sv[:, :], sv_in[:, :])
            nc.sync.dma_start(tv[:, :], tv_in[:, :])
            tqv = pool.tile([128, 4], f32, tag="tqv")
            nc.sync.dma_start(tqv[:, :], tq_in[0:1, :].partition_broadcast(128))

            mask = pool.tile([128, 2], f32, tag="mask")
            nc.sync.dma_start(mask[:, :], mask_in[:, :])
            ones_c = pool.tile([128, 1], f32, tag="ones_c")
            nc.vector.memset(ones_c[:, :], 1.0)
            ones_r = pool.tile([1, 128], f32, tag="ones_r")
            nc.vector.memset(ones_r[:, :], 1.0)

            # ---------- interleaved loads + branch contractions (per batch) ----------
            qkft_ps = psum.tile([128, NG * 8], f32, tag="qkft")
            qkc_ps = psum.tile([128, NG * 4], f32, tag="qkc")
            for b_ in range(BPC):
                g0, g1 = b_ * FR, (b_ + 1) * FR
                # loads for this batch (branch inputs only, bf16)
                for k in range(2):
                    nc.sync.dma_start(
                        xsb[k][:, b_ * NFT:(b_ + 1) * NFT],
                        xsb_in[k * 128:(k + 1) * 128, b_ * NFT:(b_ + 1) * NFT])
                nc.sync.dma_start(xct[0][:, g0 * C:g1 * C],
                                  xct_in[0:128, g0 * C:g1 * C])
                nc.sync.dma_start(xct[1][0:T - 128, g0 * C:g1 * C],
                                  xct_in[128:T, g0 * C:g1 * C])
                # ft contractions: out[(t in blk), (g,blk)*4+r]
                for g in range(g0, g1):
                    for blk in range(2):
                        m_sz = 128 if blk == 0 else T - 128
                        col0 = g * T + blk * 128
                        for kt in range(2):
                            nc.tensor.matmul(
                                qkft_ps[0:m_sz, (g * 2 + blk) * 4:(g * 2 + blk) * 4 + 4],
                                xsb[kt][:, col0:col0 + m_sz],
                                aft[kt][:, :],
                                start=(kt == 0), stop=(kt == 1))
                # c contractions: out[(c in blk), (g,blk)*2+r]
                for g in range(g0, g1):
                    for blk in range(2):
                        col0 = g * C + blk * 128
                        for kt in range(2):
                            k_sz = 128 if kt == 0 else T - 128
                            nc.tensor.matmul(
                                qkc_ps[:, (g * 2 + blk) * 2:(g * 2 + blk) * 2 + 2],
                                xct[kt][0:k_sz, col0:col0 + 128],
                                ac[kt][0:k_sz, :],
                                start=(kt == 0), stop=(kt == 1))
            qkft = pool.tile([128, NG * 8], f32, tag="qkft_sb")
            # masked eviction: zeroes t-pad rows (t=251..255) of blk1 columns
            mask_ft = mask[:, :].rearrange("p k -> p () k ()") \
                .broadcast_to([128, NG, 2, 4])
            nc.vector.tensor_tensor(
                qkft[:, :].rearrange("p (g k r) -> p g k r", k=2, r=4),
                qkft_ps[:, :].rearrange("p (g k r) -> p g k r", k=2, r=4),
                mask_ft, op=ALU.mult)
            # u offsets (tq' immediates) on u_f (col r=0) and u_t (col r=2)
            uf_all = qkft[:, :].rearrange("p (x r) -> p x r", r=4)[:, :, 0]
            ut_all = qkft[:, :].rearrange("p (x r) -> p x r", r=4)[:, :, 2]
            nc.vector.tensor_scalar_add(uf_all, uf_all, tqv[:, 0:1])
            nc.vector.tensor_scalar_add(ut_all, ut_all, tqv[:, 1:2])

            qkc = pool.tile([128, NG * 4], f32, tag="qkc_sb")
            nc.vector.tensor_copy(qkc[:, :], qkc_ps[:, :])
            uc_all = qkc[:, :].rearrange("p (x r) -> p x r", r=2)[:, :, 0]
            nc.vector.tensor_scalar_add(uc_all, uc_all, tqv[:, 2:3])

            # =======================================================
            #  branch small math.  Column conventions:
            #   qkft: col (g*2+blk)*4 + {0:u_f, 1:kv_f, 2:u_t, 3:kv_t}
            #   qkc:  col (g*2+blk)*2 + {0:u_c, 1:kv_c}
            # =======================================================
            NGB = NG * 2  # 56 (g,blk) pairs

            def qk_col(tile_, r, stride):
                return tile_[:, :].rearrange("p (x r) -> p x r", r=stride)[:, :, r]

            # ---- branches f and t, combined ----
            # qkft col = x*4 + w*2 + j   (x = (g,blk), w in {f,t}, j in {u,kv})
            qkv = qkft[:, :].rearrange("p (x w j) -> p x w j", w=2, j=2)
            u_ft = qkv[:, :, :, 0]       # [128, 56, 2]
            kv_ft = qkv[:, :, :, 1]
            powft = pool.tile([128, P * 112], f32, tag="powft")
            nc.vector.tensor_copy(
                powft[:, 0:112].rearrange("p (x w) -> p x w", w=2), kv_ft)
            for p in range(1, P):
                nc.vector.tensor_tensor(
                    powft[:, p * 112:(p + 1) * 112],
                    powft[:, (p - 1) * 112:p * 112], powft[:, 0:112], op=ALU.mult)
            # S_p^f[(p,g)]: sum over partitions and blks of the w=0 columns
            powv = powft[:, :].rearrange("q (p g k w) -> q p g k w",
                                         p=P, g=NG, k=2, w=2)
            sf_ps = psum.tile([1, P * NG], f32, tag="sml", bufs=1)
            for blk in range(2):
                nc.tensor.matmul(sf_ps[:, :], ones_c[:, :],
                                 powv[:, :, :, blk, 0],
                                 start=(blk == 0), stop=(blk == 1))
            sf_sb = pool.tile([1, P * NG], f32, tag="sf_sb")
            nc.vector.tensor_copy(sf_sb[:, :], sf_ps[:, :])
            sfw_ps = psum.tile([128, P * NG], f32, tag="wide", bufs=1)
            nc.tensor.matmul(sfw_ps[:, :], ones_r[:, :], sf_sb[:, :],
                             start=True, stop=True)
            # S_p^t[(p,b,blk)]: sum over f of the w=1 columns
            powtv = powft[:, :].rearrange("q (p b f k w) -> q p b k w f",
                                          p=P, b=BPC, f=FR, k=2, w=2)
            st = pool.tile([128, P * BPC * 2], f32, tag="st")
            nc.vector.tensor_reduce(
                st[:, :].rearrange("q (p b k) -> q p b k", p=P, b=BPC, k=2),
                powtv[:, :, :, :, 1, :], axis=AX.X, op=ALU.add)

            # Horner for f and t together: den_ft [128, 112] cols (x, w)
            den_ft = pool.tile([128, 112], f32, tag="den_ft")
            tmp_ft = pool.tile([128, 112], f32, tag="tmp_ft")
            denv = den_ft[:, :].rearrange("p (g k w) -> p g k w", g=NG, k=2, w=2)
            tmpv = tmp_ft[:, :].rearrange("p (g k w) -> p g k w", g=NG, k=2, w=2)
            stv = st[:, :].rearrange("q (p b k) -> q p b k", p=P, b=BPC, k=2)

            def f_slice(t_, blk):   # [128, 28] f-columns of one blk
                return t_[:, :, blk, 0]

            def t_slice(t_, blk):   # [128, (b, f)] t-columns of one blk
                return t_[:, :, blk, 1].rearrange("p (b f) -> p b f", b=BPC)

            def st_co(p, blk):      # [128, (b, f-bcast)]
                return stv[:, p, :, blk].rearrange("q b -> q b ()") \
                    .broadcast_to([128, BPC, FR])

            for blk in range(2):
                nc.vector.tensor_scalar_mul(
                    f_slice(denv, blk), sfw_ps[:, (P - 1) * NG:P * NG],
                    1.0 / _FACT[P])
                nc.vector.tensor_scalar_mul(
                    t_slice(denv, blk), st_co(P - 1, blk), 1.0 / _FACT[P])
            for p in range(P - 1, 0, -1):
                nc.vector.tensor_tensor(tmp_ft[:, :], den_ft[:, :],
                                        u_ft, op=ALU.mult)
                for blk in range(2):
                    nc.vector.scalar_tensor_tensor(
                        f_slice(denv, blk), sfw_ps[:, (p - 1) * NG:p * NG],
                        1.0 / _FACT[p], f_slice(tmpv, blk),
                        op0=ALU.mult, op1=ALU.add)
                    nc.vector.scalar_tensor_tensor(
                        t_slice(denv, blk), st_co(p - 1, blk),
                        1.0 / _FACT[p], t_slice(tmpv, blk),
                        op0=ALU.mult, op1=ALU.add)
            nc.vector.tensor_tensor(tmp_ft[:, :], den_ft[:, :], u_ft, op=ALU.mult)
            nc.vector.tensor_scalar_add(
                tmp_ft[:, :].rearrange("p (x w) -> p w x", w=2)[:, 0],
                tmp_ft[:, :].rearrange("p (x w) -> p w x", w=2)[:, 0], float(T))
            nc.vector.tensor_scalar_add(
                tmp_ft[:, :].rearrange("p (x w) -> p w x", w=2)[:, 1],
                tmp_ft[:, :].rearrange("p (x w) -> p w x", w=2)[:, 1], float(FR))
            # tmp_ft now holds den; ratios for f and t in one go
            zft = pool.tile([128, 112], f32, tag="zft")
            nc.vector.tensor_tensor(
                zft[:, :].rearrange("p (x w) -> p x w", w=2), u_ft, kv_ft,
                op=ALU.mult)
            numft = pool.tile([128, 112], f32, tag="numft")
            nc.scalar.activation(numft[:, :], zft[:, :], FT.Exp)
            recft = pool.tile([128, 112], f32, tag="recft")
            nc.vector.reciprocal(recft[:, :], tmp_ft[:, :])
            ratft = pool.tile([128, 112], f32, tag="ratft")
            nc.vector.tensor_tensor(ratft[:, :], numft[:, :], recft[:, :],
                                    op=ALU.mult)
            ratv = ratft[:, :].rearrange("p (g k w) -> p g k w", g=NG, k=2, w=2)

            # ---- branch c ----
            u_c = qk_col(qkc, 0, 2)
            kv_c = qk_col(qkc, 1, 2)
            powc = pool.tile([128, P * NGB], f32, tag="powc")
            nc.vector.tensor_copy(powc[:, 0:NGB], kv_c)
            for p in range(1, P):
                nc.vector.tensor_tensor(
                    powc[:, p * NGB:(p + 1) * NGB],
                    powc[:, (p - 1) * NGB:p * NGB], powc[:, 0:NGB], op=ALU.mult)
            sc_ps = psum.tile([1, P * NG], f32, tag="sml", bufs=1)
            for blk in range(2):
                rhs = powc[:, :].rearrange("q (p g b) -> q p g b", b=2, g=NG)[:, :, :, blk]
                nc.tensor.matmul(sc_ps[:, :], ones_c[:, :], rhs,
                                 start=(blk == 0), stop=(blk == 1))
            sc_sb = pool.tile([1, P * NG], f32, tag="sc_sb")
            nc.vector.tensor_copy(sc_sb[:, :], sc_ps[:, :])
            scw_ps = psum.tile([128, P * NG], f32, tag="wide", bufs=1)
            nc.tensor.matmul(scw_ps[:, :], ones_r[:, :], sc_sb[:, :],
                             start=True, stop=True)

            def sc_coeff(p, blk):
                return scw_ps[:, p * NG:(p + 1) * NG]

            den_c = pool.tile([128, NGB], f32, tag="den_c")
            tmp_c = pool.tile([128, NGB], f32, tag="tmp_c")

            def cblk(t_, blk):
                return t_[:, :].rearrange("p (x k) -> p k x", k=2)[:, blk]

            for blk in range(2):
                nc.vector.tensor_scalar_mul(
                    cblk(den_c, blk), sc_coeff(P - 1, blk), 1.0 / _FACT[P])
            for p in range(P - 1, 0, -1):
                nc.vector.tensor_tensor(tmp_c[:, :], den_c[:, :], u_c, op=ALU.mult)
                for blk in range(2):
                    nc.vector.scalar_tensor_tensor(
                        cblk(den_c, blk), sc_coeff(p - 1, blk), 1.0 / _FACT[p],
                        cblk(tmp_c, blk), op0=ALU.mult, op1=ALU.add)
            nc.vector.tensor_tensor(tmp_c[:, :], den_c[:, :], u_c, op=ALU.mult)
            nc.vector.tensor_scalar_add(den_c[:, :], tmp_c[:, :], float(C))
            z_c = pool.tile([128, NGB], f32, tag="z_c")
            nc.vector.tensor_tensor(z_c[:, :], u_c, kv_c, op=ALU.mult)
            num_c = pool.tile([128, NGB], f32, tag="num_c")
            nc.scalar.activation(num_c[:, :], z_c[:, :], FT.Exp)
            rec_c = pool.tile([128, NGB], f32, tag="rec_c")
            nc.vector.reciprocal(rec_c[:, :], den_c[:, :])
            rat_c = pool.tile([128, NGB], f32, tag="rat_c")
            nc.vector.tensor_tensor(rat_c[:, :], num_c[:, :], rec_c[:, :],
                                    op=ALU.mult)

            # df[t] partial: sum over g -> [128, 2(blk)]
            df_t = pool.tile([128, 2], f32, tag="df_t")
            nc.vector.tensor_reduce(
                df_t[:, :].rearrange("p k -> p k ()"),
                ratv[:, :, :, 0].rearrange("p g k -> p k g"),
                axis=AX.X, op=ALU.add)
            # dc[c] partial
            dc_t = pool.tile([128, 2], f32, tag="dc_t")
            nc.vector.tensor_reduce(
                dc_t[:, :].rearrange("p k -> p k ()"),
                rat_c[:, :].rearrange("p (g k) -> p k g", k=2),
                axis=AX.X, op=ALU.add)
            # dt[f] partial: reduce over b, mask t-pads, then reduce over t (partitions)
            dt_red = pool.tile([128, FR * 2], f32, tag="dt_red")
            nc.vector.tensor_reduce(
                dt_red[:, :].rearrange("p (f k) -> p f k ()", k=2),
                ratft[:, :].rearrange("p (b f k w) -> p f k w b",
                                      b=BPC, f=FR, k=2, w=2)[:, :, :, 1],
                axis=AX.X, op=ALU.add)
            nc.vector.tensor_tensor(
                dt_red[:, :].rearrange("p (f k) -> p f k", k=2),
                dt_red[:, :].rearrange("p (f k) -> p f k", k=2),
                mask[:, :].rearrange("p k -> p () k").broadcast_to([128, FR, 2]),
                op=ALU.mult)
            dtp_ps = psum.tile([1, FR * 2], f32, tag="sml", bufs=1)
            nc.tensor.matmul(dtp_ps[:, :], ones_c[:, :], dt_red[:, :],
                             start=True, stop=True)
            dtp_sb = pool.tile([1, FR * 2], f32, tag="dtp_sb")
            nc.vector.tensor_copy(dtp_sb[:, :], dtp_ps[:, :])
            dt_sb = pool.tile([1, FR], f32, tag="dt_sb")
            nc.vector.tensor_tensor(
                dt_sb[:, :],
                dtp_sb[:, :].rearrange("q (f k) -> q f k", k=2)[:, :, 0],
                dtp_sb[:, :].rearrange("q (f k) -> q f k", k=2)[:, :, 1],
                op=ALU.add)

            # ---------- allreduce of [dc(256), df(251), dt(7)] ----------
            ar_in = dram.tile([1, 544], f32)
            ar_out = dram.tile([8, 544], f32, addr_space="Shared")
            nc.sync.dma_start(
                ar_in[0:1, 0:256].rearrange("q (k p) -> q p k", k=2), dc_t[:, :])
            nc.sync.dma_start(ar_in[0:1, 256:384].rearrange("q p -> p q"),
                              df_t[:, 0:1])
            nc.sync.dma_start(ar_in[0:1, 384:507].rearrange("q p -> p q"),
                              df_t[0:123, 1:2])
            nc.sync.dma_start(ar_in[0:1, 507:514], dt_sb[:, :])
            # make the unused tail deterministic
            pad_sb = pool.tile([1, 30], f32, tag="pad_sb")
            nc.vector.memset(pad_sb[:, :], 0.0)
            nc.sync.dma_start(ar_in[0:1, 514:544], pad_sb[:, :])
            nc.gpsimd.collective_compute(
                "AllGather", ALU.bypass,
                replica_groups=[list(range(N_CORES))],
                ins=[ar_in[:, :].opt()], outs=[ar_out[:, :].opt()])

            # ---------- post-AR: build scale tiles ----------
            dc_g = pool.tile([128, 2 * 8], f32, tag="dc_g")
            for k in range(2):
                nc.sync.dma_start(
                    dc_g[:, k * 8:(k + 1) * 8],
                    ar_out[:, k * 128:(k + 1) * 128].rearrange("r p -> p r"))
            dc_col = pool.tile([128, 2], f32, tag="dc_col")
            nc.vector.tensor_reduce(
                dc_col[:, :].rearrange("p k -> p k ()"),
                dc_g[:, :].rearrange("p (k r) -> p k r", r=8),
                axis=AX.X, op=ALU.add)
            ftd_g = pool.tile([128, (T + FR) * 8], f32, tag="xsb0")
            nc.sync.dma_start(
                ftd_g[:, :].rearrange("p (r x) -> p r x", r=8),
                ar_out[:, 256:514].rearrange("r x -> () r x")
                .broadcast_to([128, 8, T + FR]))
            ftd = pool.tile([128, T + FR], f32, tag="ftd")
            nc.vector.tensor_reduce(
                ftd[:, :].rearrange("p x -> p x ()"),
                ftd_g[:, :].rearrange("p (r x) -> p x r", r=8),
                axis=AX.X, op=ALU.add)
            g_rep = pool.tile([128, NFT], f32, tag="g_rep")
            nc.vector.tensor_tensor(
                g_rep[:, :].rearrange("p (f t) -> p f t", f=FR),
                ftd[:, T:T + FR].rearrange("p f -> p f ()").broadcast_to([128, FR, T]),
                ftd[:, 0:T].rearrange("p t -> p () t").broadcast_to([128, FR, T]),
                op=ALU.add)


            # ---------- x (f32r) loads + v matmul + BN eviction (overlap the AR) ----------
            if VMM_SPLIT3:
                whi = [pool.tile([128, 128], bf16, tag=f"whi{i}", name=f"whi{i}")
                       for i in range(4)]
                wlo = [pool.tile([128, 128], bf16, tag=f"wlo{i}", name=f"wlo{i}")
                       for i in range(4)]
                xlo = [pool.tile([128, NCOLS], bf16, tag=f"xct{i}", name=f"xlo{i}")
                       for i in range(2)]
                for kt in range(2):
                    for m in range(2):
                        nc.sync.dma_start(
                            whi[kt * 2 + m][:, :],
                            whi_in[kt * 128:(kt + 1) * 128, m * 128:(m + 1) * 128])
                        nc.sync.dma_start(
                            wlo[kt * 2 + m][:, :],
                            wlo_in[kt * 128:(kt + 1) * 128, m * 128:(m + 1) * 128])
                for k in range(2):
                    for b_ in range(BPC):
                        nc.sync.dma_start(
                            xlo[k][:, b_ * NFT:(b_ + 1) * NFT],
                            xlo_in[k * 128:(k + 1) * 128,
                                   b_ * NFT:(b_ + 1) * NFT])
            else:
                for kt in range(2):
                    for m in range(2):
                        nc.sync.dma_start(
                            vwT[kt * 2 + m][:, :],
                            vwT_in[kt * 128:(kt + 1) * 128, m * 128:(m + 1) * 128])
            HX = 879
            for b_ in range(BPC):
                for k in range(2):
                    for hh in range(2):
                        cw = (NFT - HX) if hh else HX
                        nc.sync.dma_start(
                            xs[k][:, b_ * NFT + hh * HX:b_ * NFT + hh * HX + cw],
                            x_in[b_, k * 128:(k + 1) * 128, hh * HX:hh * HX + cw])
            v_sb = [pool.tile([128, NCOLS], f32, tag=f"v{m}", name=f"v{m}") for m in range(2)]
            CH = 512
            n_ch = (NCOLS + CH - 1) // CH
            for m in range(2):
                for ci in range(n_ch):
                    c0 = ci * CH
                    cw = min(CH, NCOLS - c0)
                    vp = psv.tile([128, CH], f32, tag="vch")
                    if VMM_SPLIT3:
                        terms = []
                        for kt in range(2):
                            terms += [
                                (whi[kt * 2 + m], xsb[kt]),
                                (whi[kt * 2 + m], xlo[kt]),
                                (wlo[kt * 2 + m], xsb[kt]),
                            ]
                        for ti, (lh, rh) in enumerate(terms):
                            nc.tensor.matmul(vp[:, 0:cw], lh[:, :],
                                             rh[:, c0:c0 + cw],
                                             start=(ti == 0),
                                             stop=(ti == len(terms) - 1))
                    else:
                        for kt in range(2):
                            nc.tensor.matmul(vp[:, 0:cw], vwT[kt * 2 + m][:, :],
                                             xs[kt][:, c0:c0 + cw],
                                             start=(kt == 0), stop=(kt == 1))
                    nc.scalar.activation(v_sb[m][:, c0:c0 + cw], vp[:, 0:cw],
                                         FT.Identity,
                                         bias=tv[:, m:m + 1], scale=sv[:, m:m + 1])

            # ---------- apply + store ----------
            H1 = 879
            chunks = []
            for m in range(2):
                for b_ in range(BPC):
                    for h in range(2):
                        c0 = b_ * NFT + h * H1
                        cw = (NFT - H1) if h else H1
                        chunks.append((m, b_, h, c0, cw))
            for i, (m, b_, h, c0, cw) in enumerate(chunks):
                t1 = pool.tile([128, H1], f32, tag=f"t1_{i % 6}", name=f"t1_{i}")
                add_eng = nc.gpsimd if (i % 3 != 2) else nc.vector
                nc.vector.scalar_tensor_tensor(
                    t1[:, 0:cw], g_rep[:, h * H1:h * H1 + cw], dc_col[:, m:m + 1],
                    v_sb[m][:, c0:c0 + cw], op0=ALU.add, op1=ALU.mult)
                add_eng.tensor_tensor(t1[:, 0:cw], t1[:, 0:cw],
                                      xs[m][:, c0:c0 + cw].bitcast(f32), op=ALU.add)
                nc.sync.dma_start(
                    out_d[b_, m * 128:(m + 1) * 128, h * H1:h * H1 + cw],
                    t1[:, 0:cw])

    nc.finalize()
    return nc


_NC_CACHE = None


def _get_program():
    global _NC_CACHE
    if _NC_CACHE is None:
        _NC_CACHE = _build_program()
    return _NC_CACHE


def kernel(x, qc_w, qc_bn, kc_w, kc_bn, lc_bn,
           qf_w, qf_bn, kf_w, kf_bn, lf_bn,
           qt_w, qt_bn, kt_w, kt_bn, lt_bn,
           v_w, v_b, v_bn, **_ignored):
    x = np.asarray(x, np.float32)

    # ---- fold weights on host (weight-only preprocessing) ----
    aqc, tqc, akc = _branch_fold(np.asarray(qc_w), np.asarray(qc_bn),
                                 np.asarray(kc_w), np.asarray(kc_bn),
                                 np.asarray(lc_bn))
    aqf, tqf, akf = _branch_fold(np.asarray(qf_w), np.asarray(qf_bn),
                                 np.asarray(kf_w), np.asarray(kf_bn),
                                 np.asarray(lf_bn))
    aqt, tqt, akt = _branch_fold(np.asarray(qt_w), np.asarray(qt_bn),
                                 np.asarray(kt_w), np.asarray(kt_bn),
                                 np.asarray(lt_bn))

    s_v, t_v = _bn_fold(np.asarray(v_bn))
    tv_full = (t_v + s_v * np.asarray(v_b, np.float64)).astype(np.float32)
    sv_full = s_v.astype(np.float32)

    aft = np.stack([aqf, akf, aqt, akt], axis=1).astype(ml_dtypes.bfloat16)  # [256, 4]
    ac = np.stack([aqc, akc], axis=1).astype(ml_dtypes.bfloat16)     # [251, 2]
    vwT = np.ascontiguousarray(np.asarray(v_w, np.float32).T)        # [256, 256]
    whi = vwT.astype(ml_dtypes.bfloat16)
    wlo = (vwT - whi.astype(np.float32)).astype(ml_dtypes.bfloat16)
    sv2 = np.ascontiguousarray(sv_full.reshape(2, 128).T)            # [128, 2]
    tv2 = np.ascontiguousarray(tv_full.reshape(2, 128).T)
    tqv = np.array([[tqf, tqt, tqc, 0.0]], np.float32)
    mask = np.ones((128, 2), np.float32)
    mask[123:, 1] = 0.0

    in_maps = []
    for core in range(N_CORES):
        xb = x[core * BPC:(core + 1) * BPC]                           # [4,256,7,251]
        x_slice = np.ascontiguousarray(xb.reshape(BPC, C, NFT))
        x_cn = np.ascontiguousarray(x_slice.transpose(1, 0, 2).reshape(C, NCOLS))
        xsb = x_cn.astype(ml_dtypes.bfloat16)
        xlo = (x_cn - xsb.astype(np.float32)).astype(ml_dtypes.bfloat16)
        xct = np.ascontiguousarray(
            xb.transpose(3, 0, 2, 1).reshape(T, NC_COLS)).astype(ml_dtypes.bfloat16)
        in_maps.append({
            "x_in": x_slice, "xct_in": xct, "aft_in": aft, "ac_in": ac,
            "xsb_in": xsb,
            "whi_in": whi, "wlo_in": wlo, "xlo_in": xlo,
            "vwT_in": vwT, "sv_in": sv2, "tv_in": tv2, "tqv_in": tqv,
            "mask_in": mask,
        })
    # fix key name
    for m in in_maps:
        m["tq_in"] = m.pop("tqv_in")

    nc = _get_program()
    res = run_bass_kernel_spmd(nc, in_maps, list(range(N_CORES)))

    out = np.empty((B, C, FR, T), np.float32)
    for core in range(N_CORES):
        out[core * BPC:(core + 1) * BPC] = \
            res.results[core]["out"].reshape(BPC, C, FR, T)
    return out


if __name__ == "__main__":
    rng = np.random.default_rng(0)
    x = rng.standard_normal((B, C, FR, T), np.float32)
    print("built program ok")
    _get_program()
    print("finalized ok")



# revision 6
# speedup vs baseline: 1.5871x; 1.5871x over previous
"""Trainium2 Bass kernel for nn_C_T_F_Attention_90529320665770.

Math (per reference.py):
  Each branch (c,f,t) does conv1x1+BN on q,k then means over the output
  channel axis.  BN is a per-channel affine, so
     mean_o BN(W @ x)_o = ((1/O) * s @ W) . x + mean(t)  =: a . x + tbar
  i.e. each branch's q,k collapse to a single length-Cin contraction.
  logits = outer(qv, kv); softmax over j of  u_i * kv_j  with
  u = scale_l * (qv + tbar_q); the j-constant terms cancel in softmax.
  Needed output is only the softmax diagonal summed over rows:
     d[i] = sum_n exp(u_i kv_i) / sum_j exp(u_i kv_j)
  |u_i kv_j| is small, so  sum_j exp(u_i kv_j) = sum_p u_i^p/p! * S_p,
  S_p = sum_j kv_j^p  (Taylor-moment trick, degree 4).
  Final: out = v * (dc[c] + dt[f] + df[t]) + x,  v = BN(v_w @ x + v_b).

Error budget: the output is dominated by v*(dc+dt+df) with |scale| ~ 1150
(dt alone ~ B*T/FR), output absmax ~ 5.6e3, and the harness gate is
rel_err < 2e-2.  So: the d-vectors only need ~1% accuracy (fp8
contractions are fine), v only needs bf16, and the final output can be
rounded to bf16 (halves store traffic).

Sharding: pure data-parallel over batch B=32 across 8 cores (4 each);
the only coupling is an AllGather of 640 floats (dc,df,dt partials).

Performance notes (TimelineSim cost model):
  - DMA transfers serialize on one DMA_ENGINES resource at ~360GB/s;
    every dma_start also eats ~630ns on the exclusive HWDGE generator,
    so few, large DMAs.
  - collective_compute has a flat 15us cost: issue it as early as
    possible -> load the small fp8 contraction inputs first.
  - PE matmul costs out_free_size * 0.42ns (bf16/fp8).
"""
import sys
sys.path.insert(0, '/opt/trn_rl_repo')

import contextlib

import numpy as np
import ml_dtypes

import concourse.bass as bass
import concourse.tile as tile
from concourse import bacc, mybir
from concourse.bass_utils import run_bass_kernel_spmd

EPS = 1e-5
C, FR, T = 256, 7, 251
B = 32
N_CORES = 8
BPC = B // N_CORES              # batches per core = 4
NFT = FR * T                    # 1757
NCOLS = BPC * NFT               # 7028  (b,f,t) columns per core
NG = BPC * FR                   # 28 (b,f) groups per core
NGB = NG * 2                    # 56 (g,blk) pairs
NC_COLS = NG * C                # 7168  (b,f,c) columns for branch c
P = 4                           # taylor degree
S8 = 256.0                      # fp8 coefficient pre-scale
OUT_BF16 = True                 # store output as bf16 (host converts)
CONTRACT_FP8 = True             # fp8 x copies for the qk contractions

f32 = mybir.dt.float32
bf16 = mybir.dt.bfloat16
f8 = mybir.dt.float8e4
FT = mybir.ActivationFunctionType
ALU = mybir.AluOpType
AX = mybir.AxisListType

_FACT = [1.0, 1.0, 2.0, 6.0, 24.0]

np_f8 = ml_dtypes.float8_e4m3
np_bf16 = ml_dtypes.bfloat16


def _bn_fold(bn):
    g, b_, m, v = bn.astype(np.float64)
    s = g / np.sqrt(v + EPS)
    t = b_ - m * s
    return s, t


def _branch_fold(qw, qbn, kw, kbn, lbn):
    s_q, t_q = _bn_fold(qbn)
    s_k, _ = _bn_fold(kbn)
    o = qw.shape[0]
    a_q = (s_q @ qw.astype(np.float64)) / o
    tq = t_q.mean()
    a_k = (s_k @ kw.astype(np.float64)) / o
    gl, bl, ml, vl = lbn[:, 0].astype(np.float64)
    scale_l = gl / np.sqrt(vl + EPS)
    return (scale_l * a_q).astype(np.float32), np.float32(scale_l * tq), \
        a_k.astype(np.float32)


def _build_program():
    cdt = f8 if CONTRACT_FP8 else bf16
    odt = bf16 if OUT_BF16 else f32
    nc = bacc.Bacc("TRN2", target_bir_lowering=False, debug=False,
                   num_devices=N_CORES)

    # ---- per-core DRAM I/O ----
    xq_in = nc.declare_dram_parameter("xq_in", [C, NCOLS], cdt, isOutput=False)
    xct_in = nc.declare_dram_parameter("xct_in", [T, NC_COLS], cdt,
                                       isOutput=False)
    xsb_in = nc.declare_dram_parameter("xsb_in", [C, NCOLS], bf16,
                                       isOutput=False)
    co_in = nc.declare_dram_parameter("co_in", [128, 12], cdt, isOutput=False)
    par_in = nc.declare_dram_parameter("par_in", [128, 9], f32, isOutput=False)
    vw_in = nc.declare_dram_parameter("vw_in", [128, 512], bf16,
                                      isOutput=False)
    out_d = nc.declare_dram_parameter("out", [BPC, C, NFT], odt, isOutput=True)

    with tile.TileContext(nc) as tc:
        ctx = contextlib.ExitStack()
        with ctx:
            pool = ctx.enter_context(tc.tile_pool(name="sb", bufs=1))
            psum = ctx.enter_context(tc.tile_pool(name="ps", bufs=1,
                                                  space="PSUM"))
            psv = ctx.enter_context(tc.tile_pool(name="psv", bufs=4,
                                                 space="PSUM"))
            dram = ctx.enter_context(tc.tile_pool(name="dr", bufs=1,
                                                  space="DRAM"))

            # ---------- loads (all on SP/HWDGE, program order = priority) ----
            co = pool.tile([128, 12], cdt, tag="co")
            nc.sync.dma_start(co[:, :], co_in[:, :])
            par = pool.tile([128, 9], f32, tag="par")
            nc.sync.dma_start(par[:, :], par_in[:, :])
            vw = pool.tile([128, 512], bf16, tag="vw")
            nc.sync.dma_start(vw[:, :], vw_in[:, :])

            xq = [pool.tile([128, NCOLS], cdt, tag=f"xq{k}", name=f"xq{k}")
                  for k in range(2)]
            for k in range(2):
                nc.sync.dma_start(xq[k][:, :],
                                  xq_in[k * 128:(k + 1) * 128, :])
            xct = [pool.tile([128, NC_COLS], cdt, tag=f"xct{k}",
                             name=f"xct{k}") for k in range(2)]
            nc.sync.dma_start(xct[0][:, :], xct_in[0:128, :])
            nc.sync.dma_start(xct[1][0:T - 128, :], xct_in[128:T, :])

            xsb = [pool.tile([128, NCOLS], bf16, tag=f"xsb{k}",
                             name=f"xsb{k}") for k in range(2)]
            for k in range(2):
                nc.sync.dma_start(xsb[k][:, :],
                                  xsb_in[k * 128:(k + 1) * 128, :])

            ones_c = pool.tile([128, 1], f32, tag="ones_c")
            nc.vector.memset(ones_c[:, :], 1.0)
            ones_r = pool.tile([1, 128], f32, tag="ones_r")
            nc.vector.memset(ones_r[:, :], 1.0)
            # eviction scale: mask01 / S8  (zeroes t-pad rows, undoes fp8
            # coefficient pre-scale)
            mask_ev = pool.tile([128, 2], f32, tag="mask_ev")
            nc.vector.tensor_scalar_mul(mask_ev[:, :], par[:, 7:9],
                                        1.0 / S8 if CONTRACT_FP8 else 1.0)

            # ---------- branch contractions ----------
            # qkft: col (g*2+blk)*4 + {0:u_f, 1:kv_f, 2:u_t, 3:kv_t}
            # qkc:  col (g*2+blk)*2 + {0:u_c, 1:kv_c}
            qkft_ps = psum.tile([128, NG * 8], f32, tag="qkft")
            qkc_ps = psum.tile([128, NG * 4], f32, tag="qkc")
            for g in range(NG):
                for blk in range(2):
                    m_sz = 128 if blk == 0 else T - 128
                    col0 = g * T + blk * 128
                    for kt in range(2):
                        nc.tensor.matmul(
                            qkft_ps[0:m_sz,
                                    (g * 2 + blk) * 4:(g * 2 + blk) * 4 + 4],
                            xq[kt][:, col0:col0 + m_sz],
                            co[:, kt * 4:kt * 4 + 4],
                            start=(kt == 0), stop=(kt == 1))
            for g in range(NG):
                for blk in range(2):
                    col0 = g * C + blk * 128
                    for kt in range(2):
                        k_sz = 128 if kt == 0 else T - 128
                        nc.tensor.matmul(
                            qkc_ps[:, (g * 2 + blk) * 2:(g * 2 + blk) * 2 + 2],
                            xct[kt][0:k_sz, col0:col0 + 128],
                            co[0:k_sz, 8 + kt * 2:8 + kt * 2 + 2],
                            start=(kt == 0), stop=(kt == 1))

            # ---------- evictions + u offsets ----------
            qkft = pool.tile([128, NG * 8], f32, tag="qkft_sb")
            mask_ft = mask_ev[:, :].rearrange("p k -> p () k ()") \
                .broadcast_to([128, NG, 2, 4])
            nc.vector.tensor_tensor(
                qkft[:, :].rearrange("p (g k r) -> p g k r", k=2, r=4),
                qkft_ps[:, :].rearrange("p (g k r) -> p g k r", k=2, r=4),
                mask_ft, op=ALU.mult)
            uf_all = qkft[:, :].rearrange("p (x r) -> p x r", r=4)[:, :, 0]
            ut_all = qkft[:, :].rearrange("p (x r) -> p x r", r=4)[:, :, 2]
            nc.vector.tensor_scalar_add(uf_all, uf_all, par[:, 4:5])
            nc.vector.tensor_scalar_add(ut_all, ut_all, par[:, 5:6])

            qkc = pool.tile([128, NG * 4], f32, tag="qkc_sb")
            if CONTRACT_FP8:
                nc.vector.tensor_scalar_mul(qkc[:, :], qkc_ps[:, :], 1.0 / S8)
            else:
                nc.vector.tensor_copy(qkc[:, :], qkc_ps[:, :])
            uc_all = qkc[:, :].rearrange("p (x r) -> p x r", r=2)[:, :, 0]
            nc.vector.tensor_scalar_add(uc_all, uc_all, par[:, 6:7])

            # ---------- branches f and t, combined ----------
            qkv = qkft[:, :].rearrange("p (x w j) -> p x w j", w=2, j=2)
            u_ft = qkv[:, :, :, 0]       # [128, 56, 2]
            kv_ft = qkv[:, :, :, 1]
            powft = pool.tile([128, P * 112], f32, tag="powft")
            nc.vector.tensor_copy(
                powft[:, 0:112].rearrange("p (x w) -> p x w", w=2), kv_ft)
            for p in range(1, P):
                nc.vector.tensor_tensor(
                    powft[:, p * 112:(p + 1) * 112],
                    powft[:, (p - 1) * 112:p * 112], powft[:, 0:112],
                    op=ALU.mult)
            powv = powft[:, :].rearrange("q (p g k w) -> q p g k w",
                                         p=P, g=NG, k=2, w=2)
            sf_ps = psum.tile([1, P * NG], f32, tag="sml", bufs=1)
            for blk in range(2):
                nc.tensor.matmul(sf_ps[:, :], ones_c[:, :],
                                 powv[:, :, :, blk, 0],
                                 start=(blk == 0), stop=(blk == 1))
            sf_sb = pool.tile([1, P * NG], f32, tag="sf_sb")
            nc.vector.tensor_copy(sf_sb[:, :], sf_ps[:, :])
            sfw_ps = psum.tile([128, P * NG], f32, tag="wide", bufs=1)
            nc.tensor.matmul(sfw_ps[:, :], ones_r[:, :], sf_sb[:, :],
                             start=True, stop=True)
            powtv = powft[:, :].rearrange("q (p b f k w) -> q p b k w f",
                                          p=P, b=BPC, f=FR, k=2, w=2)
            st = pool.tile([128, P * BPC * 2], f32, tag="st")
            nc.vector.tensor_reduce(
                st[:, :].rearrange("q (p b k) -> q p b k", p=P, b=BPC, k=2),
                powtv[:, :, :, :, 1, :], axis=AX.X, op=ALU.add)

            den_ft = pool.tile([128, 112], f32, tag="den_ft")
            tmp_ft = pool.tile([128, 112], f32, tag="tmp_ft")
            denv = den_ft[:, :].rearrange("p (g k w) -> p g k w",
                                          g=NG, k=2, w=2)
            tmpv = tmp_ft[:, :].rearrange("p (g k w) -> p g k w",
                                          g=NG, k=2, w=2)
            stv = st[:, :].rearrange("q (p b k) -> q p b k", p=P, b=BPC, k=2)

            def f_slice(t_, blk):
                return t_[:, :, blk, 0]

            def t_slice(t_, blk):
                return t_[:, :, blk, 1].rearrange("p (b f) -> p b f", b=BPC)

            def st_co(p, blk):
                return stv[:, p, :, blk].rearrange("q b -> q b ()") \
                    .broadcast_to([128, BPC, FR])

            for blk in range(2):
                nc.vector.tensor_scalar_mul(
                    f_slice(denv, blk), sfw_ps[:, (P - 1) * NG:P * NG],
                    1.0 / _FACT[P])
                nc.vector.tensor_scalar_mul(
                    t_slice(denv, blk), st_co(P - 1, blk), 1.0 / _FACT[P])
            for p in range(P - 1, 0, -1):
                nc.vector.tensor_tensor(tmp_ft[:, :], den_ft[:, :],
                                        u_ft, op=ALU.mult)
                for blk in range(2):
                    nc.vector.scalar_tensor_tensor(
                        f_slice(denv, blk), sfw_ps[:, (p - 1) * NG:p * NG],
                        1.0 / _FACT[p], f_slice(tmpv, blk),
                        op0=ALU.mult, op1=ALU.add)
                    nc.vector.scalar_tensor_tensor(
                        t_slice(denv, blk), st_co(p - 1, blk),
                        1.0 / _FACT[p], t_slice(tmpv, blk),
                        op0=ALU.mult, op1=ALU.add)
            nc.vector.tensor_tensor(tmp_ft[:, :], den_ft[:, :], u_ft,
                                    op=ALU.mult)
            nc.vector.tensor_scalar_add(
                tmp_ft[:, :].rearrange("p (x w) -> p w x", w=2)[:, 0],
                tmp_ft[:, :].rearrange("p (x w) -> p w x", w=2)[:, 0],
                float(T))
            nc.vector.tensor_scalar_add(
                tmp_ft[:, :].rearrange("p (x w) -> p w x", w=2)[:, 1],
                tmp_ft[:, :].rearrange("p (x w) -> p w x", w=2)[:, 1],
                float(FR))
            zft = pool.tile([128, 112], f32, tag="zft")
            nc.vector.tensor_tensor(
                zft[:, :].rearrange("p (x w) -> p x w", w=2), u_ft, kv_ft,
                op=ALU.mult)
            numft = pool.tile([128, 112], f32, tag="numft")
            nc.scalar.activation(numft[:, :], zft[:, :], FT.Exp)
            recft = pool.tile([128, 112], f32, tag="recft")
            nc.vector.reciprocal(recft[:, :], tmp_ft[:, :])
            ratft = pool.tile([128, 112], f32, tag="ratft")
            nc.vector.tensor_tensor(ratft[:, :], numft[:, :], recft[:, :],
                                    op=ALU.mult)
            ratv = ratft[:, :].rearrange("p (g k w) -> p g k w",
                                         g=NG, k=2, w=2)

            # ---------- branch c ----------
            u_c = qkc[:, :].rearrange("p (x r) -> p x r", r=2)[:, :, 0]
            kv_c = qkc[:, :].rearrange("p (x r) -> p x r", r=2)[:, :, 1]
            powc = pool.tile([128, P * NGB], f32, tag="powc")
            nc.vector.tensor_copy(powc[:, 0:NGB], kv_c)
            for p in range(1, P):
                nc.vector.tensor_tensor(
                    powc[:, p * NGB:(p + 1) * NGB],
                    powc[:, (p - 1) * NGB:p * NGB], powc[:, 0:NGB],
                    op=ALU.mult)
            sc_ps = psum.tile([1, P * NG], f32, tag="sml", bufs=1)
            for blk in range(2):
                rhs = powc[:, :].rearrange("q (p g b) -> q p g b",
                                           b=2, g=NG)[:, :, :, blk]
                nc.tensor.matmul(sc_ps[:, :], ones_c[:, :], rhs,
                                 start=(blk == 0), stop=(blk == 1))
            sc_sb = pool.tile([1, P * NG], f32, tag="sc_sb")
            nc.vector.tensor_copy(sc_sb[:, :], sc_ps[:, :])
            scw_ps = psum.tile([128, P * NG], f32, tag="wide", bufs=1)
            nc.tensor.matmul(scw_ps[:, :], ones_r[:, :], sc_sb[:, :],
                             start=True, stop=True)

            den_c = pool.tile([128, NGB], f32, tag="den_c")
            tmp_c = pool.tile([128, NGB], f32, tag="tmp_c")

            def cblk(t_, blk):
                return t_[:, :].rearrange("p (x k) -> p k x", k=2)[:, blk]

            for blk in range(2):
                nc.vector.tensor_scalar_mul(
                    cblk(den_c, blk), scw_ps[:, (P - 1) * NG:P * NG],
                    1.0 / _FACT[P])
            for p in range(P - 1, 0, -1):
                nc.vector.tensor_tensor(tmp_c[:, :], den_c[:, :], u_c,
                                        op=ALU.mult)
                for blk in range(2):
                    nc.vector.scalar_tensor_tensor(
                        cblk(den_c, blk), scw_ps[:, (p - 1) * NG:p * NG],
                        1.0 / _FACT[p], cblk(tmp_c, blk),
                        op0=ALU.mult, op1=ALU.add)
            nc.vector.tensor_tensor(tmp_c[:, :], den_c[:, :], u_c,
                                    op=ALU.mult)
            nc.vector.tensor_scalar_add(den_c[:, :], tmp_c[:, :], float(C))
            z_c = pool.tile([128, NGB], f32, tag="z_c")
            nc.vector.tensor_tensor(z_c[:, :], u_c, kv_c, op=ALU.mult)
            num_c = pool.tile([128, NGB], f32, tag="num_c")
            nc.scalar.activation(num_c[:, :], z_c[:, :], FT.Exp)
            rec_c = pool.tile([128, NGB], f32, tag="rec_c")
            nc.vector.reciprocal(rec_c[:, :], den_c[:, :])
            rat_c = pool.tile([128, NGB], f32, tag="rat_c")
            nc.vector.tensor_tensor(rat_c[:, :], num_c[:, :], rec_c[:, :],
                                    op=ALU.mult)

            # ---------- per-core partial d sums into one combined tile ------
            # combined [128, 5]: k0,k1 = dc halves; k2,k3 = df halves (junk
            # rows >=123 in k3); k4 = dt at partitions 0..6.
            comb = pool.tile([128, 5], f32, tag="comb")
            nc.vector.memset(comb[:, 4:5], 0.0)
            nc.vector.tensor_reduce(
                comb[:, 0:2].rearrange("p k -> p k ()"),
                rat_c[:, :].rearrange("p (g k) -> p k g", k=2),
                axis=AX.X, op=ALU.add)
            nc.vector.tensor_reduce(
                comb[:, 2:4].rearrange("p k -> p k ()"),
                ratv[:, :, :, 0].rearrange("p g k -> p k g"),
                axis=AX.X, op=ALU.add)
            # dt: reduce ratios over b on DVE -> [128, (k f)], then contract
            # the t-partitions against the 0/1 mask columns on PE.
            dt_red = pool.tile([128, FR * 2], f32, tag="dt_red")
            nc.vector.tensor_reduce(
                dt_red[:, :].rearrange("p (k f) -> p f k ()", k=2),
                ratft[:, :].rearrange("p (b f k w) -> p f k w b",
                                      b=BPC, f=FR, k=2, w=2)[:, :, :, 1],
                axis=AX.X, op=ALU.add)
            dt_ps = psum.tile([FR, 1], f32, tag="sml", bufs=1)
            for blk in range(2):
                nc.tensor.matmul(dt_ps[:, :],
                                 dt_red[:, blk * FR:(blk + 1) * FR],
                                 par[:, 7 + blk:8 + blk],
                                 start=(blk == 0), stop=(blk == 1))
            nc.scalar.activation(comb[0:FR, 4:5], dt_ps[:, :], FT.Identity)

            # ---------- allreduce: single DMA in, AllGather, single DMA out -
            ar_in = dram.tile([1, 640], f32)
            ar_out = dram.tile([8, 640], f32, addr_space="Shared")
            nc.sync.dma_start(
                ar_in[0:1, :].rearrange("q (p k) -> (q p) k", p=128, k=5),
                comb[:, :])
            nc.gpsimd.collective_compute(
                "AllGather", ALU.bypass,
                replica_groups=[list(range(N_CORES))],
                ins=[ar_in[:, :].opt()], outs=[ar_out[:, :].opt()])
            arg = pool.tile([8, 640], f32, tag="arg")
            nc.sync.dma_start(arg[:, :], ar_out[:, :])

            # ---------- v = BN(v_w @ x + v_b) from bf16 (overlaps the AR) ---
            v_sb = [pool.tile([128, NCOLS], bf16, tag=f"v{m}", name=f"v{m}")
                    for m in range(2)]
            CH = 512
            n_ch = (NCOLS + CH - 1) // CH
            for m in range(2):
                for ci in range(n_ch):
                    c0 = ci * CH
                    cw = min(CH, NCOLS - c0)
                    vp = psv.tile([128, CH], f32, tag="vch")
                    for kt in range(2):
                        nc.tensor.matmul(vp[:, 0:cw],
                                         vw[:, kt * 256 + m * 128:
                                            kt * 256 + m * 128 + 128],
                                         xsb[kt][:, c0:c0 + cw],
                                         start=(kt == 0), stop=(kt == 1))
                    nc.scalar.activation(v_sb[m][:, c0:c0 + cw], vp[:, 0:cw],
                                         FT.Identity,
                                         bias=par[:, 2 + m:3 + m],
                                         scale=par[:, m:m + 1])

            # ---------- post-AR: rebuild global d on-chip ----------
            # column sums over the 8 cores (PE), slot s[5p+k] layout
            sums_ps_a = psum.tile([1, 512], f32, tag="sml", bufs=1)
            nc.tensor.matmul(sums_ps_a[:, :], ones_c[0:8, :], arg[:, 0:512],
                             start=True, stop=True)
            sums_ps_b = psum.tile([1, 128], f32, tag="qkc", bufs=1)
            nc.tensor.matmul(sums_ps_b[:, :], ones_c[0:8, :], arg[:, 512:640],
                             start=True, stop=True)
            sums = pool.tile([1, 640], f32, tag="sums")
            nc.scalar.activation(sums[0:1, 0:512], sums_ps_a[:, :],
                                 FT.Identity)
            nc.scalar.activation(sums[0:1, 512:640], sums_ps_b[:, :],
                                 FT.Identity)
            sview = sums[0:1, :].rearrange("q (p k) -> q k p", k=5)

            # dc as per-partition scalars via PE transpose
            dct_ps = psum.tile([128, 2], f32, tag="wide", bufs=1)
            for k in range(2):
                nc.tensor.transpose(dct_ps[:, k:k + 1], sview[:, k, :],
                                    ones_c[0:1, 0:1])
            dc_col = pool.tile([128, 2], f32, tag="dc_col")
            nc.scalar.activation(dc_col[:, :], dct_ps[:, :], FT.Identity)

            # df/dt broadcast across partitions via PE
            bc_ps = psum.tile([128, 263], f32, tag="qkft", bufs=1)
            nc.tensor.matmul(bc_ps[:, 0:128], ones_r[:, :], sview[:, 2, :],
                             start=True, stop=True)
            nc.tensor.matmul(bc_ps[:, 128:256], ones_r[:, :], sview[:, 3, :],
                             start=True, stop=True)
            nc.tensor.matmul(bc_ps[:, 256:263], ones_r[:, :],
                             sview[:, 4, 0:FR], start=True, stop=True)
            # g_rep[p, (f,t)] = df[t] + dt[f]   (bf16 for 2x apply ops)
            g_rep = pool.tile([128, NFT], bf16, tag="g_rep")
            nc.vector.tensor_tensor(
                g_rep[:, :].rearrange("p (f t) -> p f t", f=FR),
                bc_ps[:, 256:263].rearrange("p f -> p f ()")
                .broadcast_to([128, FR, T]),
                bc_ps[:, 0:T].rearrange("p t -> p () t")
                .broadcast_to([128, FR, T]),
                op=ALU.add)
            # s_m[p, (f,t)] = g_rep + dc[c=m*128+p]
            s_m = [pool.tile([128, NFT], bf16, tag=f"s_m{m}", name=f"s_m{m}")
                   for m in range(2)]
            for m in range(2):
                nc.vector.tensor_scalar_add(s_m[m][:, :], g_rep[:, :],
                                            dc_col[:, m:m + 1])

            # ---------- apply + store ----------
            for i, (b_, m) in enumerate([(b_, m) for b_ in range(BPC)
                                         for m in range(2)]):
                sl = slice(b_ * NFT, (b_ + 1) * NFT)
                t1 = pool.tile([128, NFT], bf16, tag=f"t1_{i % 2}",
                               name=f"t1_{i}")
                t2 = pool.tile([128, NFT], odt, tag=f"t2_{i % 4}",
                               name=f"t2_{i}")
                e2 = nc.gpsimd if i % 3 == 1 else nc.vector
                nc.vector.tensor_tensor(t1[:, :], s_m[m][:, :], v_sb[m][:, sl],
                                        op=ALU.mult)
                e2.tensor_tensor(t2[:, :], t1[:, :], xsb[m][:, sl],
                                 op=ALU.add)
                nc.sync.dma_start(
                    out_d[b_, m * 128:(m + 1) * 128, :], t2[:, :])

    nc.finalize()
    return nc


_NC_CACHE = None


def _get_program():
    global _NC_CACHE
    if _NC_CACHE is None:
        _NC_CACHE = _build_program()
    return _NC_CACHE


def kernel(x, qc_w, qc_bn, kc_w, kc_bn, lc_bn,
           qf_w, qf_bn, kf_w, kf_bn, lf_bn,
           qt_w, qt_bn, kt_w, kt_bn, lt_bn,
           v_w, v_b, v_bn, **_ignored):
    x = np.asarray(x, np.float32)
    np_cdt = np_f8 if CONTRACT_FP8 else np_bf16
    cs = S8 if CONTRACT_FP8 else 1.0

    # ---- fold weights on host (weight-only preprocessing) ----
    aqc, tqc, akc = _branch_fold(np.asarray(qc_w), np.asarray(qc_bn),
                                 np.asarray(kc_w), np.asarray(kc_bn),
                                 np.asarray(lc_bn))
    aqf, tqf, akf = _branch_fold(np.asarray(qf_w), np.asarray(qf_bn),
                                 np.asarray(kf_w), np.asarray(kf_bn),
                                 np.asarray(lf_bn))
    aqt, tqt, akt = _branch_fold(np.asarray(qt_w), np.asarray(qt_bn),
                                 np.asarray(kt_w), np.asarray(kt_bn),
                                 np.asarray(lt_bn))

    s_v, t_v = _bn_fold(np.asarray(v_bn))
    tv_full = (t_v + s_v * np.asarray(v_b, np.float64)).astype(np.float32)
    sv_full = s_v.astype(np.float32)

    # co [128, 12]: cols 0:4 aft(kt=0), 4:8 aft(kt=1), 8:10 ac(p0), 10:12 ac(p1)
    aft = np.stack([aqf, akf, aqt, akt], axis=1) * cs        # [256, 4]
    ac = np.stack([aqc, akc], axis=1) * cs                   # [251, 2]
    co = np.zeros((128, 12), np.float32)
    co[:, 0:4] = aft[0:128]
    co[:, 4:8] = aft[128:256]
    co[:, 8:10] = ac[0:128]
    co[0:T - 128, 10:12] = ac[128:T]
    co = co.astype(np_cdt)

    # par [128, 9]: 0:2 sv halves, 2:4 tv halves, 4:7 tq bcast, 7:9 mask01
    par = np.zeros((128, 9), np.float32)
    par[:, 0:2] = sv_full.reshape(2, 128).T
    par[:, 2:4] = tv_full.reshape(2, 128).T
    par[:, 4] = tqf
    par[:, 5] = tqt
    par[:, 6] = tqc
    par[:, 7] = 1.0
    par[0:T - 128, 8] = 1.0

    # vw [128, 512]: [p, kt*256 + m*128 + j] = v_w[m*128+j, kt*128+p]
    vwT = np.asarray(v_w, np.float32).T                      # [cin, cout]
    vw = np.zeros((128, 512), np.float32)
    for kt in range(2):
        for m in range(2):
            vw[:, kt * 256 + m * 128:kt * 256 + m * 128 + 128] = \
                vwT[kt * 128:(kt + 1) * 128, m * 128:(m + 1) * 128]
    vw = vw.astype(np_bf16)

    in_maps = []
    for core in range(N_CORES):
        xb = x[core * BPC:(core + 1) * BPC]                  # [4,256,7,251]
        x_slice = xb.reshape(BPC, C, NFT)
        x_cn = np.ascontiguousarray(
            x_slice.transpose(1, 0, 2).reshape(C, NCOLS))
        xsb = x_cn.astype(np_bf16)
        xq = x_cn.astype(np_cdt)
        xct = np.ascontiguousarray(
            xb.transpose(3, 0, 2, 1).reshape(T, NC_COLS)).astype(np_cdt)
        in_maps.append({
            "xq_in": xq, "xct_in": xct, "xsb_in": xsb,
            "co_in": co, "par_in": par, "vw_in": vw,
        })

    nc = _get_program()
    res = run_bass_kernel_spmd(nc, in_maps, list(range(N_CORES)))

    out = np.empty((B, C, FR, T), np.float32)
    for core in range(N_CORES):
        out[core * BPC:(core + 1) * BPC] = \
            res.results[core]["out"].astype(np.float32) \
            .reshape(BPC, C, FR, T)
    return out


if __name__ == "__main__":
    print("building program ...")
    _get_program()
    print("finalized ok")


# revision 15
# speedup vs baseline: 1.6402x; 1.0334x over previous
"""Trainium2 Bass kernel for nn_C_T_F_Attention_90529320665770.

Math (per reference.py):
  Each branch (c,f,t) does conv1x1+BN on q,k then means over the output
  channel axis.  BN is a per-channel affine, so
     mean_o BN(W @ x)_o = ((1/O) * s @ W) . x + mean(t)  =: a . x + tbar
  i.e. each branch's q,k collapse to a single length-Cin contraction.
  logits = outer(qv, kv); softmax over j of  u_i * kv_j  with
  u = scale_l * (qv + tbar_q); the j-constant terms cancel in softmax.
  Needed output is only the softmax diagonal summed over rows:
     d[i] = sum_n exp(u_i kv_i) / sum_j exp(u_i kv_j)
  |u_i kv_j| is small, so  sum_j exp(u_i kv_j) = sum_p u_i^p/p! * S_p,
  S_p = sum_j kv_j^p  (Taylor-moment trick, degree 4).
  Final: out = v * (dc[c] + dt[f] + df[t]) + x,  v = BN(v_w @ x + v_b).

Error budget: the output is dominated by v*(dc+dt+df) with |scale| ~ 1150
(dt alone ~ B*T/FR), output absmax ~ 5.6e3, and the harness gate is
rel_err < 2e-2.  So: the d-vectors only need ~1% accuracy (fp8
contractions are fine), v only needs bf16, and the final output can be
rounded to bf16 (halves store traffic).

Sharding: pure data-parallel over batch B=32 across 8 cores (4 each);
the only coupling is an AllGather of 640 floats (dc,df,dt partials).

Performance notes (TimelineSim cost model):
  - DMA transfers serialize on one DMA_ENGINES resource at ~360GB/s;
    every dma_start also eats ~630ns on the exclusive HWDGE generator,
    so few, large DMAs.
  - collective_compute has a flat 15us cost: issue it as early as
    possible -> load the small fp8 contraction inputs first.
  - PE matmul costs out_free_size * 0.42ns (bf16/fp8).
"""
import sys
sys.path.insert(0, '/opt/trn_rl_repo')

import contextlib

import numpy as np
import ml_dtypes

import concourse.bass as bass
import concourse.tile as tile
from concourse import bacc, mybir
from concourse.bass_utils import run_bass_kernel_spmd

EPS = 1e-5
C, FR, T = 256, 7, 251
B = 32
N_CORES = 8
BPC = B // N_CORES              # batches per core = 4
NFT = FR * T                    # 1757
NCOLS = BPC * NFT               # 7028  (b,f,t) columns per core
NG = BPC * FR                   # 28 (b,f) groups per core
NGB = NG * 2                    # 56 (g,blk) pairs
NC_COLS = NG * C                # 7168  (b,f,c) columns for branch c
P = 4                           # taylor degree
S8 = 256.0                      # fp8 coefficient pre-scale
OUT_BF16 = True                 # store output as bf16 (host converts)
CONTRACT_FP8 = True             # fp8 x copies for the qk contractions

f32 = mybir.dt.float32
bf16 = mybir.dt.bfloat16
f8 = mybir.dt.float8e4
FT = mybir.ActivationFunctionType
ALU = mybir.AluOpType
AX = mybir.AxisListType

_FACT = [1.0, 1.0, 2.0, 6.0, 24.0]

np_f8 = ml_dtypes.float8_e4m3
np_bf16 = ml_dtypes.bfloat16


def _bn_fold(bn):
    g, b_, m, v = bn.astype(np.float64)
    s = g / np.sqrt(v + EPS)
    t = b_ - m * s
    return s, t


def _branch_fold(qw, qbn, kw, kbn, lbn):
    s_q, t_q = _bn_fold(qbn)
    s_k, _ = _bn_fold(kbn)
    o = qw.shape[0]
    a_q = (s_q @ qw.astype(np.float64)) / o
    tq = t_q.mean()
    a_k = (s_k @ kw.astype(np.float64)) / o
    gl, bl, ml, vl = lbn[:, 0].astype(np.float64)
    scale_l = gl / np.sqrt(vl + EPS)
    return (scale_l * a_q).astype(np.float32), np.float32(scale_l * tq), \
        a_k.astype(np.float32)


def _build_program():
    cdt = f8 if CONTRACT_FP8 else bf16
    odt = bf16 if OUT_BF16 else f32
    nc = bacc.Bacc("TRN2", target_bir_lowering=False, debug=False,
                   num_devices=N_CORES)

    # ---- per-core DRAM I/O ----
    xq_in = nc.declare_dram_parameter("xq_in", [C, NCOLS], cdt, isOutput=False)
    xct_in = nc.declare_dram_parameter("xct_in", [T, NC_COLS], cdt,
                                       isOutput=False)
    xsb_in = nc.declare_dram_parameter("xsb_in", [C, NCOLS], bf16,
                                       isOutput=False)
    co_in = nc.declare_dram_parameter("co_in", [128, 12], cdt, isOutput=False)
    par_in = nc.declare_dram_parameter("par_in", [128, 9], f32, isOutput=False)
    vw_in = nc.declare_dram_parameter("vw_in", [128, 640], bf16,
                                      isOutput=False)
    out_d = nc.declare_dram_parameter("out", [BPC, C, NFT], odt, isOutput=True)

    with tile.TileContext(nc) as tc:
        ctx = contextlib.ExitStack()
        with ctx:
            pool = ctx.enter_context(tc.tile_pool(name="sb", bufs=1))
            psum = ctx.enter_context(tc.tile_pool(name="ps", bufs=1,
                                                  space="PSUM"))
            psv = ctx.enter_context(tc.tile_pool(name="psv", bufs=4,
                                                 space="PSUM"))
            dram = ctx.enter_context(tc.tile_pool(name="dr", bufs=1,
                                                  space="DRAM"))

            # ---------- loads (all on SP/HWDGE, program order = priority) ----
            co = pool.tile([128, 12], cdt, tag="co")
            nc.sync.dma_start(co[:, :], co_in[:, :])
            xq = [pool.tile([128, NCOLS], cdt, tag=f"xq{k}", name=f"xq{k}")
                  for k in range(2)]
            for k in range(2):
                nc.sync.dma_start(xq[k][:, :],
                                  xq_in[k * 128:(k + 1) * 128, :])
            xct = [pool.tile([128, NC_COLS], cdt, tag=f"xct{k}",
                             name=f"xct{k}") for k in range(2)]
            nc.sync.dma_start(xct[0][:, :], xct_in[0:128, :])
            nc.sync.dma_start(xct[1][0:T - 128, :], xct_in[128:T, :])
            par = pool.tile([128, 9], f32, tag="par")
            nc.sync.dma_start(par[:, :], par_in[:, :])
            vw = pool.tile([128, 640], bf16, tag="vw")
            nc.sync.dma_start(vw[:, :], vw_in[:, :])

            xsb = [pool.tile([128, NCOLS], bf16, tag=f"xsb{k}",
                             name=f"xsb{k}") for k in range(2)]
            for k in range(2):
                nc.sync.dma_start(xsb[k][:, :],
                                  xsb_in[k * 128:(k + 1) * 128, :])

            ones_c = pool.tile([128, 1], f32, tag="ones_c")
            nc.vector.memset(ones_c[:, :], 1.0)
            ones_r = pool.tile([1, 128], f32, tag="ones_r")
            nc.vector.memset(ones_r[:, :], 1.0)
            # eviction scale: mask01 / S8  (zeroes t-pad rows, undoes fp8
            # coefficient pre-scale)
            mask_ev = pool.tile([128, 2], f32, tag="mask_ev")
            nc.vector.tensor_scalar_mul(mask_ev[:, :], par[:, 7:9],
                                        1.0 / S8 if CONTRACT_FP8 else 1.0)

            # ---------- branch contractions ----------
            # qkft: col (g*2+blk)*4 + {0:u_f, 1:kv_f, 2:u_t, 3:kv_t}
            # qkc:  col (g*2+blk)*2 + {0:u_c, 1:kv_c}
            qkft_ps = psum.tile([128, NG * 8], f32, tag="qkft")
            qkc_ps = psum.tile([128, NG * 4], f32, tag="qkc")
            for g in range(NG):
                for blk in range(2):
                    m_sz = 128 if blk == 0 else T - 128
                    col0 = g * T + blk * 128
                    for kt in range(2):
                        nc.tensor.matmul(
                            qkft_ps[0:m_sz,
                                    (g * 2 + blk) * 4:(g * 2 + blk) * 4 + 4],
                            xq[kt][:, col0:col0 + m_sz],
                            co[:, kt * 4:kt * 4 + 4],
                            start=(kt == 0), stop=(kt == 1))
            for g in range(NG):
                for blk in range(2):
                    col0 = g * C + blk * 128
                    for kt in range(2):
                        k_sz = 128 if kt == 0 else T - 128
                        nc.tensor.matmul(
                            qkc_ps[:, (g * 2 + blk) * 2:(g * 2 + blk) * 2 + 2],
                            xct[kt][0:k_sz, col0:col0 + 128],
                            co[0:k_sz, 8 + kt * 2:8 + kt * 2 + 2],
                            start=(kt == 0), stop=(kt == 1))

            # ---------- evictions + u offsets ----------
            qkft = pool.tile([128, NG * 8], f32, tag="qkft_sb")
            mask_ft = mask_ev[:, :].rearrange("p k -> p () k ()") \
                .broadcast_to([128, NG, 2, 4])
            nc.vector.tensor_tensor(
                qkft[:, :].rearrange("p (g k r) -> p g k r", k=2, r=4),
                qkft_ps[:, :].rearrange("p (g k r) -> p g k r", k=2, r=4),
                mask_ft, op=ALU.mult)
            uf_all = qkft[:, :].rearrange("p (x r) -> p x r", r=4)[:, :, 0]
            ut_all = qkft[:, :].rearrange("p (x r) -> p x r", r=4)[:, :, 2]
            nc.vector.tensor_scalar_add(uf_all, uf_all, par[:, 4:5])
            nc.vector.tensor_scalar_add(ut_all, ut_all, par[:, 5:6])

            qkc = pool.tile([128, NG * 4], f32, tag="qkc_sb")
            if CONTRACT_FP8:
                nc.vector.tensor_scalar_mul(qkc[:, :], qkc_ps[:, :], 1.0 / S8)
            else:
                nc.vector.tensor_copy(qkc[:, :], qkc_ps[:, :])
            uc_all = qkc[:, :].rearrange("p (x r) -> p x r", r=2)[:, :, 0]
            nc.vector.tensor_scalar_add(uc_all, uc_all, par[:, 6:7])

            # ---------- degree-1 Taylor denominators ----------
            # den = L + u*S1 (|z| < 0.07 makes the quadratic term ~2e-4 rel,
            # far below the error budget).
            qkv = qkft[:, :].rearrange("p (x w j) -> p x w j", w=2, j=2)
            u_ft = qkv[:, :, :, 0]       # [128, 56, 2]
            kv_ft = qkv[:, :, :, 1]
            u_c = qkc[:, :].rearrange("p (x r) -> p x r", r=2)[:, :, 0]
            kv_c = qkc[:, :].rearrange("p (x r) -> p x r", r=2)[:, :, 1]

            # S1_f[g], S1_c[g]: column sums over partitions + blocks (PE)
            s1_ps = psum.tile([1, 2 * NG], f32, tag="sml", bufs=1)
            kvfv = qkft[:, :].rearrange("p (g k w j) -> p g k w j",
                                        g=NG, k=2, w=2, j=2)
            for blk in range(2):
                nc.tensor.matmul(s1_ps[:, 0:NG], ones_c[:, :],
                                 kvfv[:, :, blk, 0, 1],
                                 start=(blk == 0), stop=(blk == 1))
            kvcv = qkc[:, :].rearrange("p (g k r) -> p g k r", g=NG, k=2, r=2)
            for blk in range(2):
                nc.tensor.matmul(s1_ps[:, NG:2 * NG], ones_c[:, :],
                                 kvcv[:, :, blk, 1],
                                 start=(blk == 0), stop=(blk == 1))
            s1_sb = pool.tile([1, 2 * NG], f32, tag="s1_sb")
            nc.scalar.activation(s1_sb[:, :], s1_ps[:, :], FT.Identity)
            # broadcast to all partitions, duplicating across blk
            s1w_ps = psum.tile([128, 2 * NGB], f32, tag="wide", bufs=1)
            s1v = s1_sb[0:1, :].rearrange("q (b g) -> q b g", b=2)
            for h in range(2):
                nc.tensor.matmul(
                    s1w_ps[:, h * NGB:(h + 1) * NGB], ones_r[:, :],
                    s1v[:, h, :].rearrange("q g -> q g ()")
                    .broadcast_to([1, NG, 2]),
                    start=True, stop=True)
            # S1_t[t-row, b]: per-row sums over f (DVE)
            st1 = pool.tile([128, BPC * 2], f32, tag="st1")
            nc.vector.tensor_reduce(
                st1[:, :].rearrange("q (b k) -> q b k ()", b=BPC, k=2),
                kv_ft[:, :, 1].rearrange("p (b f k) -> p b k f",
                                         b=BPC, f=FR, k=2),
                axis=AX.X, op=ALU.add)

            # dens: f and c from broadcast rows, t from per-row sums
            den_ft = pool.tile([128, 112], f32, tag="den_ft")
            denv = den_ft[:, :].rearrange("p (g k w) -> p g k w",
                                          g=NG, k=2, w=2)
            nc.vector.tensor_tensor(
                denv[:, :, :, 0].rearrange("p g k -> p (g k)"),
                s1w_ps[:, 0:NGB],
                u_ft[:, :, 0].rearrange("p x -> p x"),
                op=ALU.mult)
            st1v = st1[:, :].rearrange("q (b k) -> q b k", b=BPC)
            nc.vector.tensor_tensor(
                denv[:, :, :, 1].rearrange("p (b f) k -> p b f k", b=BPC),
                st1v[:, :, :].rearrange("q b k -> q b () k")
                .broadcast_to([128, BPC, FR, 2]),
                u_ft[:, :, 1].rearrange("p (b f k) -> p b f k",
                                        b=BPC, f=FR, k=2),
                op=ALU.mult)
            den_c = pool.tile([128, NGB], f32, tag="den_c")
            nc.vector.tensor_tensor(
                den_c[:, :], s1w_ps[:, NGB:2 * NGB],
                u_c.rearrange("p x -> p x"), op=ALU.mult)
            nc.vector.tensor_scalar_add(
                den_ft[:, :].rearrange("p (x w) -> p w x", w=2)[:, 0],
                den_ft[:, :].rearrange("p (x w) -> p w x", w=2)[:, 0],
                float(T))
            nc.vector.tensor_scalar_add(
                den_ft[:, :].rearrange("p (x w) -> p w x", w=2)[:, 1],
                den_ft[:, :].rearrange("p (x w) -> p w x", w=2)[:, 1],
                float(FR))
            nc.vector.tensor_scalar_add(den_c[:, :], den_c[:, :], float(C))

            # ratios = exp(u*kv) / den
            zft = pool.tile([128, 112], f32, tag="zft")
            nc.vector.tensor_tensor(
                zft[:, :].rearrange("p (x w) -> p x w", w=2), u_ft, kv_ft,
                op=ALU.mult)
            z_c = pool.tile([128, NGB], f32, tag="z_c")
            nc.vector.tensor_tensor(z_c[:, :], u_c, kv_c, op=ALU.mult)
            numft = pool.tile([128, 112], f32, tag="numft")
            nc.scalar.activation(numft[:, :], zft[:, :], FT.Exp)
            num_c = pool.tile([128, NGB], f32, tag="num_c")
            nc.scalar.activation(num_c[:, :], z_c[:, :], FT.Exp)
            recft = pool.tile([128, 112], f32, tag="recft")
            nc.vector.reciprocal(recft[:, :], den_ft[:, :])
            rec_c = pool.tile([128, NGB], f32, tag="rec_c")
            nc.vector.reciprocal(rec_c[:, :], den_c[:, :])
            ratft = pool.tile([128, 112], f32, tag="ratft")
            nc.vector.tensor_tensor(ratft[:, :], numft[:, :], recft[:, :],
                                    op=ALU.mult)
            rat_c = pool.tile([128, NGB], f32, tag="rat_c")
            nc.vector.tensor_tensor(rat_c[:, :], num_c[:, :], rec_c[:, :],
                                    op=ALU.mult)
            ratv = ratft[:, :].rearrange("p (g k w) -> p g k w",
                                         g=NG, k=2, w=2)

            # ---------- per-core partial d sums into one combined tile ------
            # combined [128, 5]: k0,k1 = dc halves; k2,k3 = df halves (junk
            # rows >=123 in k3); k4 = dt at partitions 0..6.
            comb = pool.tile([128, 5], f32, tag="comb")
            nc.vector.memset(comb[:, 4:5], 0.0)
            nc.vector.tensor_reduce(
                comb[:, 0:2].rearrange("p k -> p k ()"),
                rat_c[:, :].rearrange("p (g k) -> p k g", k=2),
                axis=AX.X, op=ALU.add)
            nc.vector.tensor_reduce(
                comb[:, 2:4].rearrange("p k -> p k ()"),
                ratv[:, :, :, 0].rearrange("p g k -> p k g"),
                axis=AX.X, op=ALU.add)
            # dt: reduce ratios over b on DVE -> [128, (k f)], then contract
            # the t-partitions against the 0/1 mask columns on PE.
            dt_red = pool.tile([128, FR * 2], f32, tag="dt_red")
            nc.vector.tensor_reduce(
                dt_red[:, :].rearrange("p (k f) -> p f k ()", k=2),
                ratft[:, :].rearrange("p (b f k w) -> p f k w b",
                                      b=BPC, f=FR, k=2, w=2)[:, :, :, 1],
                axis=AX.X, op=ALU.add)
            dt_ps = psum.tile([FR, 1], f32, tag="sml", bufs=1)
            for blk in range(2):
                nc.tensor.matmul(dt_ps[:, :],
                                 dt_red[:, blk * FR:(blk + 1) * FR],
                                 par[:, 7 + blk:8 + blk],
                                 start=(blk == 0), stop=(blk == 1))
            nc.scalar.activation(comb[0:FR, 4:5], dt_ps[:, :], FT.Identity)

            # ---------- allreduce: single DMA in, AllGather, single DMA out -
            ar_in = dram.tile([1, 640], f32)
            ar_out = dram.tile([8, 640], f32, addr_space="Shared")
            nc.sync.dma_start(
                ar_in[0:1, :].rearrange("q (p k) -> (q p) k", p=128, k=5),
                comb[:, :])
            nc.gpsimd.collective_compute(
                "AllGather", ALU.bypass,
                replica_groups=[list(range(N_CORES))],
                ins=[ar_in[:, :].opt()], outs=[ar_out[:, :].opt()])
            arg = pool.tile([8, 640], f32, tag="arg")
            nc.sync.dma_start(arg[:, :], ar_out[:, :])

            # ---------- v = BN(v_w @ x + v_b) from bf16 (overlaps the AR) ---
            v_sb = [pool.tile([128, NCOLS], bf16, tag=f"v{m}", name=f"v{m}")
                    for m in range(2)]
            CH = 512
            n_ch = (NCOLS + CH - 1) // CH
            for m in range(2):
                for ci in range(n_ch):
                    c0 = ci * CH
                    cw = min(CH, NCOLS - c0)
                    vp = psv.tile([128, CH], f32, tag="vch")
                    for kt in range(2):
                        nc.tensor.matmul(vp[:, 0:cw],
                                         vw[:, kt * 256 + m * 128:
                                            kt * 256 + m * 128 + 128],
                                         xsb[kt][:, c0:c0 + cw],
                                         start=(kt == 0), stop=(kt == 1))
                    nc.scalar.activation(v_sb[m][:, c0:c0 + cw], vp[:, 0:cw],
                                         FT.Identity,
                                         bias=par[:, 2 + m:3 + m],
                                         scale=par[:, m:m + 1])

            # ---------- post-AR: rebuild global d on-chip ----------
            # column sums over the 8 cores (PE), slot s[5p+k] layout
            sums_ps_a = psum.tile([1, 512], f32, tag="sml", bufs=1)
            nc.tensor.matmul(sums_ps_a[:, :], ones_c[0:8, :], arg[:, 0:512],
                             start=True, stop=True)
            sums_ps_b = psum.tile([1, 128], f32, tag="qkc", bufs=1)
            nc.tensor.matmul(sums_ps_b[:, :], ones_c[0:8, :], arg[:, 512:640],
                             start=True, stop=True)
            sums = pool.tile([1, 640], f32, tag="sums")
            nc.scalar.activation(sums[0:1, 0:512], sums_ps_a[:, :],
                                 FT.Identity)
            nc.scalar.activation(sums[0:1, 512:640], sums_ps_b[:, :],
                                 FT.Identity)
            sview = sums[0:1, :].rearrange("q (p k) -> q k p", k=5)

            # dc as per-partition scalars via PE transpose
            dct_ps = psum.tile([128, 2], f32, tag="wide", bufs=1)
            for k in range(2):
                nc.tensor.transpose(dct_ps[:, k:k + 1], sview[:, k, :],
                                    ones_c[0:1, 0:1])
            dc_col = pool.tile([128, 2], f32, tag="dc_col")
            nc.scalar.activation(dc_col[:, :], dct_ps[:, :], FT.Identity)

            # df/dt broadcast across partitions via PE
            bc_ps = psum.tile([128, 263], f32, tag="qkft", bufs=1)
            nc.tensor.matmul(bc_ps[:, 0:128], ones_r[:, :], sview[:, 2, :],
                             start=True, stop=True)
            nc.tensor.matmul(bc_ps[:, 128:256], ones_r[:, :], sview[:, 3, :],
                             start=True, stop=True)
            nc.tensor.matmul(bc_ps[:, 256:263], ones_r[:, :],
                             sview[:, 4, 0:FR], start=True, stop=True)
            # g_rep[p, (f,t)] = df[t] + dt[f]   (bf16 for 2x apply ops)
            g_rep = pool.tile([128, NFT], bf16, tag="g_rep")
            nc.vector.tensor_tensor(
                g_rep[:, :].rearrange("p (f t) -> p f t", f=FR),
                bc_ps[:, 256:263].rearrange("p f -> p f ()")
                .broadcast_to([128, FR, T]),
                bc_ps[:, 0:T].rearrange("p t -> p () t")
                .broadcast_to([128, FR, T]),
                op=ALU.add)

            # ---------- apply + store ----------
            # t1 = (g_rep + dc)*v (STT), t2 = t1 + x; spread the elementwise
            # work across DVE / Pool / (PE identity-accumulate + Act evict).
            op1_eng = [0, 1, 0, 1, 0, 1, 0, 1]   # 0=DVE 1=Pool
            op2_eng = [0, 2, 2, 2, 0, 2, 0, 2]   # 0=DVE 2=PE+Act
            for i, (b_, m) in enumerate([(b_, m) for b_ in range(BPC)
                                         for m in range(2)]):
                sl = slice(b_ * NFT, (b_ + 1) * NFT)
                t1 = pool.tile([128, NFT], bf16, tag=f"t1_{i % 4}",
                               name=f"t1_{i}")
                e1 = nc.vector if op1_eng[i] == 0 else nc.gpsimd
                e1.scalar_tensor_tensor(t1[:, :], g_rep[:, :],
                                        dc_col[:, m:m + 1], v_sb[m][:, sl],
                                        op0=ALU.add, op1=ALU.mult)
                t2 = pool.tile([128, NFT], odt, tag=f"t2_{i % 4}",
                               name=f"t2_{i}")
                if op2_eng[i] == 0:
                    nc.vector.tensor_tensor(t2[:, :], t1[:, :], xsb[m][:, sl],
                                            op=ALU.add)
                else:
                    # residual add on PE: psum = I@t1 + I@x, Act evicts
                    for h in range(4):
                        h0 = h * 512
                        hw = min(512, NFT - h0)
                        rp = psv.tile([128, 512], f32, tag="vch",
                                      name=f"rp{i}_{h}")
                        nc.tensor.matmul(rp[:, 0:hw], vw[:, 512:640],
                                         t1[:, h0:h0 + hw],
                                         start=True, stop=False)
                        nc.tensor.matmul(rp[:, 0:hw], vw[:, 512:640],
                                         xsb[m][:, b_ * NFT + h0:
                                                b_ * NFT + h0 + hw],
                                         start=False, stop=True)
                        nc.scalar.activation(t2[:, h0:h0 + hw],
                                             rp[:, 0:hw], FT.Identity)
                nc.sync.dma_start(
                    out_d[b_, m * 128:(m + 1) * 128, :], t2[:, :])

    nc.finalize()
    return nc


_NC_CACHE = None


def _get_program():
    global _NC_CACHE
    if _NC_CACHE is None:
        _NC_CACHE = _build_program()
    return _NC_CACHE


def kernel(x, qc_w, qc_bn, kc_w, kc_bn, lc_bn,
           qf_w, qf_bn, kf_w, kf_bn, lf_bn,
           qt_w, qt_bn, kt_w, kt_bn, lt_bn,
           v_w, v_b, v_bn, **_ignored):
    x = np.asarray(x, np.float32)
    np_cdt = np_f8 if CONTRACT_FP8 else np_bf16
    cs = S8 if CONTRACT_FP8 else 1.0

    # ---- fold weights on host (weight-only preprocessing) ----
    aqc, tqc, akc = _branch_fold(np.asarray(qc_w), np.asarray(qc_bn),
                                 np.asarray(kc_w), np.asarray(kc_bn),
                                 np.asarray(lc_bn))
    aqf, tqf, akf = _branch_fold(np.asarray(qf_w), np.asarray(qf_bn),
                                 np.asarray(kf_w), np.asarray(kf_bn),
                                 np.asarray(lf_bn))
    aqt, tqt, akt = _branch_fold(np.asarray(qt_w), np.asarray(qt_bn),
                                 np.asarray(kt_w), np.asarray(kt_bn),
                                 np.asarray(lt_bn))

    s_v, t_v = _bn_fold(np.asarray(v_bn))
    tv_full = (t_v + s_v * np.asarray(v_b, np.float64)).astype(np.float32)
    sv_full = s_v.astype(np.float32)

    # co [128, 12]: cols 0:4 aft(kt=0), 4:8 aft(kt=1), 8:10 ac(p0), 10:12 ac(p1)
    aft = np.stack([aqf, akf, aqt, akt], axis=1) * cs        # [256, 4]
    ac = np.stack([aqc, akc], axis=1) * cs                   # [251, 2]
    co = np.zeros((128, 12), np.float32)
    co[:, 0:4] = aft[0:128]
    co[:, 4:8] = aft[128:256]
    co[:, 8:10] = ac[0:128]
    co[0:T - 128, 10:12] = ac[128:T]
    co = co.astype(np_cdt)

    # par [128, 9]: 0:2 sv halves, 2:4 tv halves, 4:7 tq bcast, 7:9 mask01
    par = np.zeros((128, 9), np.float32)
    par[:, 0:2] = sv_full.reshape(2, 128).T
    par[:, 2:4] = tv_full.reshape(2, 128).T
    par[:, 4] = tqf
    par[:, 5] = tqt
    par[:, 6] = tqc
    par[:, 7] = 1.0
    par[0:T - 128, 8] = 1.0

    # vw [128, 640]: [p, kt*256 + m*128 + j] = v_w[m*128+j, kt*128+p];
    # cols 512:640 = identity (for PE residual adds)
    vwT = np.asarray(v_w, np.float32).T                      # [cin, cout]
    vw = np.zeros((128, 640), np.float32)
    for kt in range(2):
        for m in range(2):
            vw[:, kt * 256 + m * 128:kt * 256 + m * 128 + 128] = \
                vwT[kt * 128:(kt + 1) * 128, m * 128:(m + 1) * 128]
    vw[:, 512:640] = np.eye(128, dtype=np.float32)
    vw = vw.astype(np_bf16)

    in_maps = []
    for core in range(N_CORES):
        xb = x[core * BPC:(core + 1) * BPC]                  # [4,256,7,251]
        x_slice = xb.reshape(BPC, C, NFT)
        x_cn = np.ascontiguousarray(
            x_slice.transpose(1, 0, 2).reshape(C, NCOLS))
        xsb = x_cn.astype(np_bf16)
        xq = x_cn.astype(np_cdt)
        xct = np.ascontiguousarray(
            xb.transpose(3, 0, 2, 1).reshape(T, NC_COLS)).astype(np_cdt)
        in_maps.append({
            "xq_in": xq, "xct_in": xct, "xsb_in": xsb,
            "co_in": co, "par_in": par, "vw_in": vw,
        })

    nc = _get_program()
    res = run_bass_kernel_spmd(nc, in_maps, list(range(N_CORES)))

    out = np.empty((B, C, FR, T), np.float32)
    for core in range(N_CORES):
        out[core * BPC:(core + 1) * BPC] = \
            res.results[core]["out"].astype(np.float32) \
            .reshape(BPC, C, FR, T)
    return out


if __name__ == "__main__":
    print("building program ...")
    _get_program()
    print("finalized ok")
